# revision 5
# baseline (speedup 1.0000x reference)
"""AdditiveAttention2D (Bahdanau-style) on 8 Trainium2 NeuronCores.

Reference (per batch b):
    sW = s @ W, hU = h @ U                              [L, D]
    scores[l, m] = sum_d v[d] * tanh(sW[l, d] + hU[m, d])
    attn = softmax_m(scores);  out = attn @ h           [L, D]

Sharding: the B*L = 1024 query rows split across 8 cores (128 rows each,
each core's rows inside one batch). Each core gets its batch's full h
(keys/values) plus replicated W, U, v. No collectives; the host
concatenates the per-core output shards.

Per-core layout: d lives on partitions. For query q the tanh argument is
hU_T[d, m] + sW_T[d, q] — a per-partition-scalar broadcast add (DVE, 2x
fp32 mode), a bulk fused tanh (ScalarE — the bottleneck engine, ~1
elem/lane/cycle), and the v-weighted d-reduction as a PE matmul with v
embedded in column j of a [D, 16] stationary tile so query j lands in
PSUM partition j. Softmax skips max-subtraction (|scores| <= ||v||_1 ~ 9
since |tanh| < 1, so exp cannot overflow fp32) and uses exp's accum_out
for the row sums. exp-scores go through a PE transpose to become the
stationary operand of the attn @ h accumulation.
"""

from contextlib import ExitStack

import numpy as np

import concourse.bass as bass
import concourse.mybir as mybir
import concourse.tile as tile
from concourse import bacc
from concourse.bass_utils import run_bass_kernel_spmd
from concourse.masks import make_identity

F32 = mybir.dt.float32
BF16 = mybir.dt.bfloat16
AF = mybir.ActivationFunctionType

B, L, D = 2, 512, 128
N_CORES = 8
QPC = B * L // N_CORES  # query rows per core (128)
G = 16                  # queries per chunk (softmax granularity)
H = G // 2              # tanh sub-tile (keeps PE fed mid-chunk)
NCH = QPC // G          # chunks per core (8)
MT = L // 128           # 128-row key tiles per batch (4)


def build_nc() -> bass.Bass:
    # Bacc (not plain Bass): its compile() runs move_matmul_waits_to_ldweights
    # + generate_event_semaphores, which legalize multi-sem waits down to the
    # 1-wait-per-instruction limit this walrus enforces.
    nc = bacc.Bacc()
    s_d = nc.declare_dram_parameter("s", [QPC, D], F32, isOutput=False)
    h_d = nc.declare_dram_parameter("h", [L, D], F32, isOutput=False)
    W_d = nc.declare_dram_parameter("W", [D, D], F32, isOutput=False)
    U_d = nc.declare_dram_parameter("U", [D, D], F32, isOutput=False)
    v_d = nc.declare_dram_parameter("v", [D, 1], F32, isOutput=False)
    o_d = nc.declare_dram_parameter("out", [QPC, D], F32, isOutput=True)

    with ExitStack() as ctx:
        tc = ctx.enter_context(tile.TileContext(nc))
        consts = ctx.enter_context(tc.tile_pool(name="consts", bufs=1))
        xpool = ctx.enter_context(tc.tile_pool(name="x", bufs=3))
        tpool = ctx.enter_context(tc.tile_pool(name="t", bufs=3))
        spool = ctx.enter_context(tc.tile_pool(name="small", bufs=3))

        # ---------------- prologue ----------------
        ident = consts.tile([128, 128], F32)
        make_identity(nc, ident)
        ident16 = consts.tile([G, G], BF16)
        make_identity(nc, ident16)

        s_sb = consts.tile([QPC, D], F32)
        nc.gpsimd.dma_start(out=s_sb, in_=s_d[:, :])
        h_sb = consts.tile([128, MT, D], F32)
        nc.gpsimd.dma_start(out=h_sb, in_=h_d.rearrange("(t p) d -> p t d", p=128))
        W_sb = consts.tile([D, D], F32)
        nc.gpsimd.dma_start(out=W_sb, in_=W_d[:, :])
        U_sb = consts.tile([D, D], F32)
        nc.gpsimd.dma_start(out=U_sb, in_=U_d[:, :])
        v32 = consts.tile([D, 1], F32)
        nc.gpsimd.dma_start(out=v32, in_=v_d[:, :])

        hT_sb = consts.tile([D, MT, 128], F32)
        sT_sb = consts.tile([D, QPC], F32)
        hU_sb = consts.tile([D, L], F32)
        sW_sb = consts.tile([D, QPC], F32)
        hb_sb = consts.tile([128, MT, D], BF16)
        v_bf = consts.tile([D, 1], BF16)
        vmat = consts.tile([D, G, G], BF16)

        with tc.tile_pool(name="pp_pro", bufs=2, space="PSUM") as pp_pro:
            for t in range(MT):
                pt = pp_pro.tile([128, 128], F32, tag="pro")
                nc.tensor.transpose(pt, h_sb[:, t, :], ident)
                nc.vector.tensor_copy(hT_sb[:, t, :], pt)
            st = pp_pro.tile([128, 128], F32, tag="pro")
            nc.tensor.transpose(st, s_sb, ident)
            nc.vector.tensor_copy(sT_sb, st)

            # hU_T[dout, m] = sum_din U[din, dout] * hT[din, m]
            hU_ps = pp_pro.tile([D, L], F32, tag="pro2")
            nc.tensor.matmul(hU_ps, U_sb, hT_sb, start=True, stop=True)
            nc.vector.tensor_copy(hU_sb, hU_ps)
            sW_ps = pp_pro.tile([D, QPC], F32, tag="pro")
            nc.tensor.matmul(sW_ps, W_sb, sT_sb, start=True, stop=True)
            nc.vector.tensor_copy(sW_sb, sW_ps)

        nc.vector.tensor_copy(hb_sb, h_sb)
        nc.vector.tensor_copy(v_bf, v32)
        nc.gpsimd.memset(vmat, 0.0)
        for j in range(G):
            nc.gpsimd.tensor_copy(vmat[:, j, j : j + 1], v_bf)

        pp = ctx.enter_context(tc.tile_pool(name="pp", bufs=2, space="PSUM"))

        # ---------------- main loop (software-pipelined emission) --------
        # Engine orders we want (c = chunk):
        #   ACT: tanh(0) tanh(1) exp(0) tanh(2) exp(1) ... tanh(c+1) exp(c)
        #   DVE: adds(0) adds(1) adds(2) tail(0) adds(3) tail(1) ...
        #   PE : mms(0) mms(1) mms(2) transp/attn(0) mms(3) transp/attn(1)
        sc_tiles: dict[int, object] = {}
        exp_tiles: dict[int, object] = {}
        sum_tiles: dict[int, object] = {}

        def stage_a(c):
            """adds + tanh + v-reduction matmuls for chunk c."""
            X = xpool.tile([D, G, L], F32, tag="X")
            for j in range(G):
                q = c * G + j
                nc.vector.tensor_scalar_add(X[:, j, :], hU_sb, sW_sb[:, q : q + 1])
            T = tpool.tile([D, G, L], BF16, tag="T")
            sc = pp.tile([G, L], F32, tag="scores")
            for half in range(2):
                j0 = half * H
                nc.scalar.activation(
                    T[:, j0 : j0 + H, :], X[:, j0 : j0 + H, :], AF.Tanh
                )
                for j in range(j0, j0 + H):
                    nc.tensor.matmul(
                        sc,
                        vmat[:, j, :],
                        T[:, j, :],
                        start=(j == 0),
                        stop=(j == G - 1),
                    )
            sc_tiles[c] = sc

        def stage_exp(c):
            """exp (+ row sums) for chunk c — emitted after tanh(c+1)."""
            exp_sb = spool.tile([G, L], BF16, tag="exp")
            sums = spool.tile([G, 1], F32, tag="sums")
            nc.scalar.activation(exp_sb, sc_tiles.pop(c), AF.Exp, accum_out=sums)
            exp_tiles[c] = exp_sb
            sum_tiles[c] = sums

        def stage_tail(c):
            """softmax-normalize + attn @ h + store for chunk c."""
            recip = spool.tile([G, 1], F32, tag="recip")
            nc.vector.reciprocal(recip, sum_tiles.pop(c))
            exp_sb = exp_tiles.pop(c)
            eT_ps = pp.tile([128, MT, G], BF16, tag="eT")
            for t in range(MT):
                nc.tensor.transpose(
                    eT_ps[:, t, :], exp_sb[:, t * 128 : (t + 1) * 128], ident16
                )
            eT_sb = spool.tile([128, MT, G], BF16, tag="eTs")
            nc.vector.tensor_copy(eT_sb, eT_ps)
            at_ps = pp.tile([G, D], F32, tag="attn")
            for t in range(MT):
                nc.tensor.matmul(
                    at_ps,
                    eT_sb[:, t, :],
                    hb_sb[:, t, :],
                    start=(t == 0),
                    stop=(t == MT - 1),
                )
            out_sb = spool.tile([G, D], F32, tag="out")
            nc.vector.tensor_scalar_mul(out_sb, at_ps, recip[:, 0:1])
            nc.sync.dma_start(out=o_d[bass.ts(c, G), :], in_=out_sb)

        stage_a(0)
        if NCH > 1:
            stage_a(1)
        for c in range(NCH):
            stage_exp(c)
            if c + 2 < NCH:
                stage_a(c + 2)
            stage_tail(c)

    nc.compile()
    return nc


_NC_CACHE: list = []


def _get_nc() -> bass.Bass:
    if not _NC_CACHE:
        _NC_CACHE.append(build_nc())
    return _NC_CACHE[0]


def _make_in_maps(s, h, W, U, v):
    s2 = np.ascontiguousarray(np.asarray(s, np.float32).reshape(B * L, D))
    h2 = np.asarray(h, np.float32)
    W2 = np.ascontiguousarray(np.asarray(W, np.float32))
    U2 = np.ascontiguousarray(np.asarray(U, np.float32))
    v2 = np.ascontiguousarray(np.asarray(v, np.float32))
    in_maps = []
    for c in range(N_CORES):
        in_maps.append(
            {
                "s": np.ascontiguousarray(s2[c * QPC : (c + 1) * QPC]),
                "h": np.ascontiguousarray(h2[c * QPC // L]),
                "W": W2,
                "U": U2,
                "v": v2,
            }
        )
    return in_maps


def run_spmd(s, h, W, U, v, **kwargs):
    """Run the kernel on 8 cores; returns the BassKernelResults."""
    nc = _get_nc()
    in_maps = _make_in_maps(s, h, W, U, v)
    return run_bass_kernel_spmd(nc, in_maps, core_ids=list(range(N_CORES)), **kwargs)


def kernel(s, h, W, U, v):
    res = run_spmd(s, h, W, U, v)
    shards = [np.asarray(res.results[c]["out"]) for c in range(N_CORES)]
    return np.concatenate(shards, axis=0).reshape(B, L, D).astype(np.float32)


# revision 7
# speedup vs baseline: 1.1986x; 1.1986x over previous
"""AdditiveAttention2D (Bahdanau-style) on 8 Trainium2 NeuronCores.

Reference (per batch b):
    sW = s @ W, hU = h @ U                              [L, D]
    scores[l, m] = sum_d v[d] * tanh(sW[l, d] + hU[m, d])
    attn = softmax_m(scores);  out = attn @ h           [L, D]

Sharding: the B*L = 1024 query rows split across 8 cores (128 rows each,
each core's rows inside one batch). Each core gets its batch's full h
(keys/values) plus replicated W, U, v. No collectives; the host
concatenates the per-core output shards.

Per-core layout: d lives on partitions. For query q the tanh argument is
hU_T[d, m] + sW_T[d, q] — a per-partition-scalar broadcast add (DVE, 2x
fp32 mode), a bulk fused tanh (ScalarE — the bottleneck engine, ~1
elem/lane/cycle), and the v-weighted d-reduction as a PE matmul with v
embedded in column j of a [D, 16] stationary tile so query j lands in
PSUM partition j. Softmax skips max-subtraction (|scores| <= ||v||_1 ~ 9
since |tanh| < 1, so exp cannot overflow fp32) and uses exp's accum_out
for the row sums. exp-scores go through a PE transpose to become the
stationary operand of the attn @ h accumulation.
"""

from contextlib import ExitStack

import numpy as np

import concourse.bass as bass
import concourse.mybir as mybir
import concourse.tile as tile
from concourse import bacc
from concourse.bass_utils import run_bass_kernel_spmd
from concourse.masks import make_identity

F32 = mybir.dt.float32
BF16 = mybir.dt.bfloat16
AF = mybir.ActivationFunctionType

B, L, D = 2, 512, 128
N_CORES = 8
QPC = B * L // N_CORES  # query rows per core (128)
G = 16                  # queries per chunk (softmax granularity)
H = G // 2              # tanh sub-tile (keeps PE fed mid-chunk)
NCH = QPC // G          # chunks per core (8)
MT = L // 128           # 128-row key tiles per batch (4)


def build_nc() -> bass.Bass:
    # Bacc (not plain Bass): its compile() runs move_matmul_waits_to_ldweights
    # + generate_event_semaphores, which legalize multi-sem waits down to the
    # 1-wait-per-instruction limit this walrus enforces.
    nc = bacc.Bacc()
    s_d = nc.declare_dram_parameter("s", [QPC, D], F32, isOutput=False)
    h_d = nc.declare_dram_parameter("h", [L, D], F32, isOutput=False)
    W_d = nc.declare_dram_parameter("W", [D, D], F32, isOutput=False)
    U_d = nc.declare_dram_parameter("U", [D, D], F32, isOutput=False)
    v_d = nc.declare_dram_parameter("v", [D, 1], F32, isOutput=False)
    o_d = nc.declare_dram_parameter("out", [QPC, D], F32, isOutput=True)

    with ExitStack() as ctx:
        tc = ctx.enter_context(tile.TileContext(nc))
        consts = ctx.enter_context(tc.tile_pool(name="consts", bufs=1))
        xpool = ctx.enter_context(tc.tile_pool(name="x", bufs=3))
        tpool = ctx.enter_context(tc.tile_pool(name="t", bufs=3))
        spool = ctx.enter_context(tc.tile_pool(name="small", bufs=3))

        # ---------------- prologue ----------------
        ident = consts.tile([128, 128], F32)
        make_identity(nc, ident)
        ident16 = consts.tile([G, G], BF16)
        make_identity(nc, ident16)

        s_sb = consts.tile([QPC, D], F32)
        nc.gpsimd.dma_start(out=s_sb, in_=s_d[:, :])
        h_sb = consts.tile([128, MT, D], F32)
        nc.gpsimd.dma_start(out=h_sb, in_=h_d.rearrange("(t p) d -> p t d", p=128))
        W_sb = consts.tile([D, D], F32)
        nc.gpsimd.dma_start(out=W_sb, in_=W_d[:, :])
        U_sb = consts.tile([D, D], F32)
        nc.gpsimd.dma_start(out=U_sb, in_=U_d[:, :])
        v32 = consts.tile([D, 1], F32)
        nc.gpsimd.dma_start(out=v32, in_=v_d[:, :])

        hT_sb = consts.tile([D, MT, 128], F32)
        sT_sb = consts.tile([D, QPC], F32)
        hU_sb = consts.tile([D, L], BF16)
        sW_sb = consts.tile([D, QPC], F32)
        hb_sb = consts.tile([128, MT, D], BF16)
        v_bf = consts.tile([D, 1], BF16)
        vmat = consts.tile([D, G, G], BF16)

        with tc.tile_pool(name="pp_pro", bufs=2, space="PSUM") as pp_pro:
            for t in range(MT):
                pt = pp_pro.tile([128, 128], F32, tag="pro")
                nc.tensor.transpose(pt, h_sb[:, t, :], ident)
                nc.vector.tensor_copy(hT_sb[:, t, :], pt)
            st = pp_pro.tile([128, 128], F32, tag="pro")
            nc.tensor.transpose(st, s_sb, ident)
            nc.vector.tensor_copy(sT_sb, st)

            # hU_T[dout, m] = sum_din U[din, dout] * hT[din, m]
            hU_ps = pp_pro.tile([D, L], F32, tag="pro2")
            nc.tensor.matmul(hU_ps, U_sb, hT_sb, start=True, stop=True)
            nc.vector.tensor_copy(hU_sb, hU_ps)
            sW_ps = pp_pro.tile([D, QPC], F32, tag="pro")
            nc.tensor.matmul(sW_ps, W_sb, sT_sb, start=True, stop=True)
            nc.vector.tensor_copy(sW_sb, sW_ps)

        nc.vector.tensor_copy(hb_sb, h_sb)
        nc.vector.tensor_copy(v_bf, v32)
        nc.gpsimd.memset(vmat, 0.0)
        for j in range(G):
            nc.gpsimd.tensor_copy(vmat[:, j, j : j + 1], v_bf)

        pp = ctx.enter_context(tc.tile_pool(name="pp", bufs=2, space="PSUM"))

        # ---------------- main loop (software-pipelined emission) --------
        # Engine orders we want (c = chunk):
        #   ACT: tanh(0) tanh(1) exp(0) tanh(2) exp(1) ... tanh(c+1) exp(c)
        #   DVE: adds(0) adds(1) adds(2) tail(0) adds(3) tail(1) ...
        #   PE : mms(0) mms(1) mms(2) transp/attn(0) mms(3) transp/attn(1)
        sc_tiles: dict[int, object] = {}
        exp_tiles: dict[int, object] = {}
        sum_tiles: dict[int, object] = {}

        def stage_a(c):
            """adds + tanh + v-reduction matmuls for chunk c."""
            X = xpool.tile([D, G, L], BF16, tag="X")
            for j in range(G):
                q = c * G + j
                nc.vector.tensor_scalar_add(X[:, j, :], hU_sb, sW_sb[:, q : q + 1])
            T = tpool.tile([D, G, L], BF16, tag="T")
            sc = pp.tile([G, L], F32, tag="scores")
            for half in range(2):
                j0 = half * H
                nc.scalar.activation(
                    T[:, j0 : j0 + H, :].rearrange("p a b -> p (a b)"),
                    X[:, j0 : j0 + H, :].rearrange("p a b -> p (a b)"),
                    AF.Tanh,
                )
                for j in range(j0, j0 + H):
                    nc.tensor.matmul(
                        sc,
                        vmat[:, j, :],
                        T[:, j, :],
                        start=(j == 0),
                        stop=(j == G - 1),
                    )
            sc_tiles[c] = sc

        def stage_exp(c):
            """exp (+ row sums) for chunk c — emitted after tanh(c+1)."""
            exp_sb = spool.tile([G, L], BF16, tag="exp")
            sums = spool.tile([G, 1], F32, tag="sums")
            nc.scalar.activation(exp_sb, sc_tiles.pop(c), AF.Exp, accum_out=sums)
            exp_tiles[c] = exp_sb
            sum_tiles[c] = sums

        def stage_tail(c):
            """softmax-normalize + attn @ h + store for chunk c."""
            recip = spool.tile([G, 1], F32, tag="recip")
            nc.vector.reciprocal(recip, sum_tiles.pop(c))
            exp_sb = exp_tiles.pop(c)
            eT_ps = pp.tile([128, MT, G], BF16, tag="eT")
            for t in range(MT):
                nc.tensor.transpose(
                    eT_ps[:, t, :], exp_sb[:, t * 128 : (t + 1) * 128], ident16
                )
            eT_sb = spool.tile([128, MT, G], BF16, tag="eTs")
            nc.vector.tensor_copy(eT_sb, eT_ps)
            at_ps = pp.tile([G, D], F32, tag="attn")
            for t in range(MT):
                nc.tensor.matmul(
                    at_ps,
                    eT_sb[:, t, :],
                    hb_sb[:, t, :],
                    start=(t == 0),
                    stop=(t == MT - 1),
                )
            out_sb = spool.tile([G, D], F32, tag="out")
            nc.vector.tensor_scalar_mul(out_sb, at_ps, recip[:, 0:1])
            nc.sync.dma_start(out=o_d[bass.ts(c, G), :], in_=out_sb)

        stage_a(0)
        if NCH > 1:
            stage_a(1)
        for c in range(NCH):
            stage_exp(c)
            if c + 2 < NCH:
                stage_a(c + 2)
            stage_tail(c)

    nc.compile()
    return nc


_NC_CACHE: list = []


def _get_nc() -> bass.Bass:
    if not _NC_CACHE:
        _NC_CACHE.append(build_nc())
    return _NC_CACHE[0]


def _make_in_maps(s, h, W, U, v):
    s2 = np.ascontiguousarray(np.asarray(s, np.float32).reshape(B * L, D))
    h2 = np.asarray(h, np.float32)
    W2 = np.ascontiguousarray(np.asarray(W, np.float32))
    U2 = np.ascontiguousarray(np.asarray(U, np.float32))
    v2 = np.ascontiguousarray(np.asarray(v, np.float32))
    in_maps = []
    for c in range(N_CORES):
        in_maps.append(
            {
                "s": np.ascontiguousarray(s2[c * QPC : (c + 1) * QPC]),
                "h": np.ascontiguousarray(h2[c * QPC // L]),
                "W": W2,
                "U": U2,
                "v": v2,
            }
        )
    return in_maps


def run_spmd(s, h, W, U, v, **kwargs):
    """Run the kernel on 8 cores; returns the BassKernelResults."""
    nc = _get_nc()
    in_maps = _make_in_maps(s, h, W, U, v)
    return run_bass_kernel_spmd(nc, in_maps, core_ids=list(range(N_CORES)), **kwargs)


def kernel(s, h, W, U, v):
    res = run_spmd(s, h, W, U, v)
    shards = [np.asarray(res.results[c]["out"]) for c in range(N_CORES)]
    return np.concatenate(shards, axis=0).reshape(B, L, D).astype(np.float32)
